# revision 3
# baseline (speedup 1.0000x reference)
"""SAGEConv (mean aggregation) + ReLU on 8 Trainium2 NeuronCores.

Problem: h = relu(mean_agg(x, edges) @ W_l.T + b_l + x @ W_r.T)
  x [8, 55296, 64] f32, 221184 random edges, W [256, 64].

Strategy (dst-sharded, all-batch):
  Core c owns destination nodes [c*6912, (c+1)*6912) for ALL 8 batches.
  x is re-laid host-side as node-major rows of 512 (8 batches x 64 feats),
  cast to bf16, split into lo/hi halves (int16 gather-index limit).
  Per core:
    - dma_gather (GPSIMD mlp library) fetches per-edge source rows (1024B)
      in dst-group order -> edge-major msgs tiles [128 edges/chunk, 512] bf16.
    - Selection matrices S[e, d] = (dstloc[e] == d) built on DVE per chunk;
      TensorE computes aggT[feat, dst] = msgs^T @ S with PSUM accumulation
      over chunks (feat-major aggregation -> no transposes anywhere).
    - Self rows (x of own dsts, batch-pair-swapped on host) flow through the
      same path via plain DMA + identity-S matmuls -> xT in PSUM with
      batch-parity-swapped layout.
    - PSUM->SBUF copies assemble combined lhsT tiles [aggT_b ; xT_b] with
      partition-aligned copies (agg scaled by 1/deg on the way).
    - Phase B: one K=128 bf16 matmul per (128 dsts, batch) against stacked
      [W_l;W_r] (parity-swapped variant for odd batches), relu on DVE/ACT,
      contiguous DMA to the per-core output slice.
  Output: concat core slices -> [8, 55296, 256] f32.
"""

import numpy as np

N_NODES = 55296
F_IN = 64
F_HID = 256
BATCH = 8
NCORE = 8
ND = N_NODES // NCORE          # 6912 dsts per core
GSZ = 256                      # dst group size
NG = ND // GSZ                 # 27 groups per core
SB_G = 3                       # groups per superblock
NSB = NG // SB_G               # 9 superblocks
HALF = N_NODES // 2            # 27648
EW = BATCH * F_IN              # 512 elems per node row

_cache = {}


def _build(schedule, has_bias):
    import concourse.bacc as bacc
    import concourse.tile as tile
    import concourse.mybir as mybir
    from concourse.library_config import mlp

    KA, KB = schedule  # tuples of NG ints: chunk counts per (group, half)
    bf16 = mybir.dt.bfloat16
    f32 = mybir.dt.float32

    sb_cols = []
    for s in range(NSB):
        gs = range(s * SB_G, (s + 1) * SB_G)
        sb_cols.append((sum(KA[g] for g in gs), sum(KB[g] for g in gs)))
    tot_cols = sum(a + b for a, b in sb_cols)
    max_sb_cols = max(a + b for a, b in sb_cols)
    tot_idx = tot_cols * 128
    max_s_live = max(KA[g] + KB[g] for g in range(NG)) + 2

    nc = bacc.Bacc(None, target_bir_lowering=False, debug=False)
    with tile.TileContext(nc) as tc:
        with tc.tile_pool(name="dram", bufs=1, space="DRAM") as dram:
            xab_lo = dram.tile([HALF + 1, EW], bf16, kind="ExternalInput")
            xab_hi = dram.tile([HALF + 1, EW], bf16, kind="ExternalInput")
            xself = dram.tile([ND, EW], bf16, kind="ExternalInput")
            gidx = dram.tile([128, tot_idx // 16], mybir.dt.int16, kind="ExternalInput")
            dstloc = dram.tile([128, tot_cols], f32, kind="ExternalInput")
            selfloc = dram.tile([128, 2], f32, kind="ExternalInput")
            iota_rep = dram.tile([128, GSZ], f32, kind="ExternalInput")
            invdeg_rep = dram.tile([128, ND], f32, kind="ExternalInput")
            w_ev = dram.tile([128, F_HID], bf16, kind="ExternalInput")
            w_od = dram.tile([128, F_HID], bf16, kind="ExternalInput")
            if has_bias:
                bias_rep = dram.tile([128, F_HID], f32, kind="ExternalInput")
            out = dram.tile([BATCH, ND, F_HID], f32, kind="ExternalOutput")

            with (
                tc.tile_pool(name="const", bufs=1) as constp,
                tc.tile_pool(name="msgs", bufs=2) as msgsp,
                tc.tile_pool(name="spool", bufs=max_s_live + 2) as spool,
                tc.tile_pool(name="comb", bufs=2) as combp,
                tc.tile_pool(name="hsb", bufs=4) as hsbp,
                tc.tile_pool(name="aggps", bufs=2, space="PSUM") as aggpsp,
                tc.tile_pool(name="hps", bufs=3, space="PSUM") as hpsp,
            ):
                nc.gpsimd.load_library(mlp)

                gidx_t = constp.tile([128, tot_idx // 16], mybir.dt.int16)
                nc.sync.dma_start(out=gidx_t[:], in_=gidx[:])
                dstloc_t = constp.tile([128, tot_cols], f32)
                nc.sync.dma_start(out=dstloc_t[:], in_=dstloc[:])
                selfloc_t = constp.tile([128, 2], f32)
                nc.sync.dma_start(out=selfloc_t[:], in_=selfloc[:])
                iota_t = constp.tile([128, GSZ], f32)
                nc.sync.dma_start(out=iota_t[:], in_=iota_rep[:])
                invdeg_t = constp.tile([128, ND], f32)
                nc.sync.dma_start(out=invdeg_t[:], in_=invdeg_rep[:])
                w_ev_t = constp.tile([128, F_HID], bf16)
                nc.sync.dma_start(out=w_ev_t[:], in_=w_ev[:])
                w_od_t = constp.tile([128, F_HID], bf16)
                nc.sync.dma_start(out=w_od_t[:], in_=w_od[:])
                if has_bias:
                    bias_t = constp.tile([128, F_HID], f32)
                    nc.sync.dma_start(out=bias_t[:], in_=bias_rep[:])

                col_off = 0
                idx_off = 0
                relu_flip = 0
                for s in range(NSB):
                    acols, bcols = sb_cols[s]
                    ncols = acols + bcols
                    gs = list(range(s * SB_G, (s + 1) * SB_G))
                    m_t = msgsp.tile([128, (max_sb_cols + 2 * SB_G) * EW], bf16,
                                     tag="msgs")
                    m3 = m_t[:].rearrange("p (c e) -> p c e", e=EW)
                    for (xsrc, c0, cn) in ((xab_lo, 0, acols),
                                           (xab_hi, acols, bcols)):
                        if cn == 0:
                            continue
                        nidx = cn * 128
                        nc.gpsimd.dma_gather(
                            out_ap=m3[:, c0:c0 + cn, :],
                            in_ap=xsrc[:],
                            idxs_ap=gidx_t[:, idx_off // 16: (idx_off + nidx) // 16],
                            num_idxs=nidx,
                            num_idxs_reg=nidx,
                            elem_size=EW,
                            single_packet=False,
                        )
                        idx_off += nidx
                    for gl, g in enumerate(gs):
                        sc = ncols + 2 * gl
                        nc.sync.dma_start(
                            out=m3[:, sc:sc + 2, :],
                            in_=xself[g * GSZ:(g + 1) * GSZ, :].rearrange(
                                "(c p) e -> p c e", p=128),
                        )

                    comb = [[combp.tile([128, SB_G * GSZ], bf16,
                                        tag=f"comb{par}{fc}",
                                        name=f"comb{par}{fc}")
                             for fc in range(4)] for par in range(2)]

                    a_off = 0
                    b_off = acols
                    for gl, g in enumerate(gs):
                        cols = ([a_off + i for i in range(KA[g])] +
                                [b_off + i for i in range(KB[g])])
                        a_off += KA[g]
                        b_off += KB[g]
                        nchunk = len(cols)
                        s_tiles = []
                        for cc in cols:
                            s_t = spool.tile([128, GSZ], bf16, tag="sel")
                            nc.vector.tensor_tensor(
                                out=s_t[:],
                                in0=iota_t[:],
                                in1=dstloc_t[:, col_off + cc:col_off + cc + 1]
                                .to_broadcast([128, GSZ]),
                                op=mybir.AluOpType.is_equal,
                            )
                            s_tiles.append(s_t)
                        sself_tiles = []
                        for k in range(2):
                            s_t = spool.tile([128, GSZ], bf16, tag="sel")
                            nc.vector.tensor_tensor(
                                out=s_t[:],
                                in0=iota_t[:],
                                in1=selfloc_t[:, k:k + 1].to_broadcast([128, GSZ]),
                                op=mybir.AluOpType.is_equal,
                            )
                            sself_tiles.append(s_t)

                        dsl = slice(gl * GSZ, (gl + 1) * GSZ)
                        ivd = invdeg_t[:, g * GSZ:(g + 1) * GSZ]
                        for fc in range(4):
                            agg_ps = aggpsp.tile([128, GSZ], f32, tag="agg")
                            for ci, cc in enumerate(cols):
                                nc.tensor.matmul(
                                    out=agg_ps[:],
                                    lhsT=m3[:, cc, fc * 128:(fc + 1) * 128],
                                    rhs=s_tiles[ci][:],
                                    start=(ci == 0),
                                    stop=(ci == nchunk - 1),
                                )
                            xts_ps = aggpsp.tile([128, GSZ], f32, tag="xts")
                            for k in range(2):
                                sc = ncols + 2 * gl + k
                                nc.tensor.matmul(
                                    out=xts_ps[:],
                                    lhsT=m3[:, sc, fc * 128:(fc + 1) * 128],
                                    rhs=sself_tiles[k][:],
                                    start=(k == 0),
                                    stop=(k == 1),
                                )
                            # even batch 2fc: agg parts 0:64, x parts 64:128
                            nc.vector.tensor_mul(
                                out=comb[0][fc][:64, dsl],
                                in0=agg_ps[:64, :], in1=ivd[:64, :])
                            nc.scalar.activation(
                                out=comb[0][fc][64:128, dsl],
                                in_=xts_ps[64:128, :],
                                func=mybir.ActivationFunctionType.Copy)
                            # odd batch 2fc+1: x parts 0:64, agg parts 64:128
                            nc.scalar.activation(
                                out=comb[1][fc][:64, dsl],
                                in_=xts_ps[:64, :],
                                func=mybir.ActivationFunctionType.Copy)
                            nc.vector.tensor_mul(
                                out=comb[1][fc][64:128, dsl],
                                in0=agg_ps[64:128, :], in1=ivd[64:128, :])
                    col_off += ncols

                    for b in range(BATCH):
                        fc, par = b // 2, b % 2
                        w_t = w_od_t if par else w_ev_t
                        for dch in range(SB_G * GSZ // 128):
                            h_ps = hpsp.tile([128, F_HID], f32, tag="hps")
                            nc.tensor.matmul(
                                out=h_ps[:],
                                lhsT=comb[par][fc][:, dch * 128:(dch + 1) * 128],
                                rhs=w_t[:],
                                start=True,
                                stop=True,
                            )
                            if has_bias:
                                nc.vector.tensor_add(
                                    out=h_ps[:], in0=h_ps[:], in1=bias_t[:])
                            h_t = hsbp.tile([128, F_HID], f32, tag="hsb")
                            if relu_flip % 3 == 0:
                                nc.scalar.activation(
                                    out=h_t[:], in_=h_ps[:],
                                    func=mybir.ActivationFunctionType.Relu)
                            else:
                                nc.vector.tensor_relu(out=h_t[:], in_=h_ps[:])
                            relu_flip += 1
                            r0 = s * SB_G * GSZ + dch * 128
                            nc.sync.dma_start(
                                out=out[b, r0:r0 + 128, :], in_=h_t[:])
    nc.compile()
    names = dict(
        xab_lo=xab_lo.name, xab_hi=xab_hi.name, xself=xself.name,
        gidx=gidx.name, dstloc=dstloc.name, selfloc=selfloc.name,
        iota_rep=iota_rep.name, invdeg_rep=invdeg_rep.name,
        w_ev=w_ev.name, w_od=w_od.name, out=out.name,
        bias_rep=(bias_rep.name if has_bias else None),
    )
    return nc, names


def _prep(x, edge_src, edge_dst, W_l, b_l, W_r):
    from ml_dtypes import bfloat16

    deg = np.bincount(edge_dst, minlength=N_NODES).astype(np.float32)
    invdeg = (1.0 / np.maximum(deg, 1.0)).astype(np.float32)

    xn = np.ascontiguousarray(x.transpose(1, 0, 2)).reshape(N_NODES, EW)
    xn_bf = xn.astype(bfloat16)
    zrow = np.zeros((1, EW), dtype=bfloat16)
    xab_lo = np.ascontiguousarray(np.vstack([xn_bf[:HALF], zrow]))
    xab_hi = np.ascontiguousarray(np.vstack([xn_bf[HALF:], zrow]))

    # batch-pair swapped feature order for the self rows
    swap = np.arange(EW).reshape(BATCH, F_IN)
    swap = swap.reshape(4, 2, F_IN)[:, ::-1, :].reshape(EW)

    core = edge_dst // ND
    per_core = []
    counts = np.zeros((NCORE, NG, 2), np.int64)
    for c in range(NCORE):
        sel = core == c
        ed = (edge_dst[sel] - c * ND).astype(np.int64)
        es = edge_src[sel].astype(np.int64)
        g = ed // GSZ
        h = (es >= HALF).astype(np.int64)
        order = np.lexsort((es, h, g))
        ed, es, g, h = ed[order], es[order], g[order], h[order]
        key = g * 2 + h
        bounds = np.searchsorted(key, np.arange(2 * NG + 1))
        cnt = np.diff(bounds).reshape(NG, 2)
        counts[c] = cnt
        per_core.append((ed, es, bounds))

    K = np.ceil(counts.max(axis=0) / 128).astype(np.int64)
    K = np.maximum(K, 1)
    KA = tuple(int(v) for v in K[:, 0])
    KB = tuple(int(v) for v in K[:, 1])

    # canonical column order: per sb, A cols of its groups then B cols
    col_group = []
    for s in range(NSB):
        gs = range(s * SB_G, (s + 1) * SB_G)
        for g in gs:
            col_group += [(g, 0)] * KA[g]
        for g in gs:
            col_group += [(g, 1)] * KB[g]
    tot_cols = len(col_group)
    gh_cols = {}
    for ci, gh in enumerate(col_group):
        gh_cols.setdefault(gh, []).append(ci)

    iota_rep = np.broadcast_to(
        np.arange(GSZ, dtype=np.float32)[None, :], (128, GSZ)).copy()
    selfloc = np.stack([np.arange(128, dtype=np.float32),
                        np.arange(128, 256, dtype=np.float32)], axis=1).copy()

    WlT = W_l.T.astype(np.float32)
    WrT = W_r.T.astype(np.float32)
    w_ev = np.vstack([WlT, WrT]).astype(bfloat16)
    w_od = np.vstack([WrT, WlT]).astype(bfloat16)
    has_bias = bool(np.any(b_l != 0))
    bias_rep = (np.broadcast_to(b_l.astype(np.float32)[None, :],
                                (128, F_HID)).copy() if has_bias else None)

    in_maps = []
    for c in range(NCORE):
        ed, es, bounds = per_core[c]
        slotvals = np.full((tot_cols, 128), HALF, dtype=np.int16)
        dl = np.full((tot_cols, 128), -1.0, dtype=np.float32)
        for gg in range(NG):
            for hh in range(2):
                lo, hi = bounds[2 * gg + hh], bounds[2 * gg + hh + 1]
                cnt = hi - lo
                cols = gh_cols[(gg, hh)]
                buf = np.full(len(cols) * 128, HALF, np.int16)
                dbuf = np.full(len(cols) * 128, -1.0, np.float32)
                if cnt:
                    buf[:cnt] = (es[lo:hi] - (HALF if hh else 0)).astype(np.int16)
                    dbuf[:cnt] = (ed[lo:hi] - gg * GSZ).astype(np.float32)
                for j, ci in enumerate(cols):
                    slotvals[ci] = buf[j * 128:(j + 1) * 128]
                    dl[ci] = dbuf[j * 128:(j + 1) * 128]
        chunks = []
        ci = 0
        for s in range(NSB):
            gs = range(s * SB_G, (s + 1) * SB_G)
            na = sum(KA[g2] for g2 in gs)
            nb = sum(KB[g2] for g2 in gs)
            for cn in (na, nb):
                if cn:
                    sl = slotvals[ci:ci + cn].reshape(-1)
                    chunks.append(np.tile(sl.reshape(-1, 16).T, (8, 1)))
                    ci += cn
        gidx_arr = np.ascontiguousarray(np.concatenate(chunks, axis=1))

        invdeg_c = np.broadcast_to(
            invdeg[c * ND:(c + 1) * ND][None, :], (128, ND)).copy()
        xself_c = np.ascontiguousarray(xn_bf[c * ND:(c + 1) * ND][:, swap])

        in_maps.append(dict(
            xab_lo=xab_lo, xab_hi=xab_hi, xself=xself_c,
            gidx=gidx_arr, dstloc=np.ascontiguousarray(dl.T),
            selfloc=selfloc, iota_rep=iota_rep, invdeg_rep=invdeg_c,
            w_ev=w_ev, w_od=w_od, bias_rep=bias_rep,
        ))
    return (KA, KB), has_bias, in_maps


def kernel(x, edge_src, edge_dst, W_l, b_l, W_r):
    from concourse.bass_utils import run_bass_kernel_spmd

    x = np.asarray(x, dtype=np.float32)
    edge_src = np.asarray(edge_src, dtype=np.int32)
    edge_dst = np.asarray(edge_dst, dtype=np.int32)
    W_l = np.asarray(W_l, dtype=np.float32)
    b_l = np.asarray(b_l, dtype=np.float32)
    W_r = np.asarray(W_r, dtype=np.float32)

    schedule, has_bias, in_maps = _prep(x, edge_src, edge_dst, W_l, b_l, W_r)
    key = (schedule, has_bias)
    if key not in _cache:
        _cache[key] = _build(schedule, has_bias)
    nc, names = _cache[key]

    run_maps = []
    for m in in_maps:
        rm = {names[k]: v for k, v in m.items()
              if names.get(k) is not None and v is not None}
        run_maps.append(rm)
    res = run_bass_kernel_spmd(nc, run_maps, list(range(NCORE)))
    outs = [res.results[c][names["out"]] for c in range(NCORE)]
    return np.concatenate(outs, axis=1)
